# revision 17
# baseline (speedup 1.0000x reference)
"""Trainium2 Bass kernel for a 3-layer GCN (CityAgglomerationGNN).

Strategy (graph/data parallel over 8 NeuronCores):
  - Nodes are ranked by in-degree (desc) and dealt round-robin to cores, so
    every core owns npc = N/8 nodes with matching degree profiles. Within a
    core, node ranks are dealt round-robin over NT tiles so every 128-row
    tile (and hence every window) carries a balanced edge load.
  - GCN normalization dinv_i*dinv_j is folded into per-partition scalar
    multiplies: table rows are written as dinv*h, aggregates scaled by dinv.
  - The global table layout interleaves chunks: chunk c holds local slots
    [c*SPC,(c+1)*SPC) of EVERY core, so each chunk can be AllGathered as
    soon as its tiles' posts complete (pipelined per-chunk collectives).
  - Aggregation runs per dst window (GW tiles): per chunk, in-edge messages
    are fetched with dma_gather (int16 indices into the <=32K-row chunk) and
    reduced into per-tile PSUM ranges via one-hot matmuls. The one-hot S
    blocks are generated ON-CHIP by the (otherwise idle) vector engine:
    S[e, c] = (spv[e, j] == c) via tensor_tensor is_equal over broadcast
    APs, from a tiny fp16 spv table (128 x NMM) - no S streaming from HBM.
  - The self-loop term is an identity matmul against the core's own table.
  - Post chain per tile: agg*dinv -> PE transpose -> relu(x + b) -> matmul
    with the next layer's weights -> *dinv -> next table block.
  - SPMD: one Bass program for all 8 cores; per-(window,chunk) segment
    lengths are equalized to the max over cores (pad indices gather row 0;
    their S column is zero via spv=-1).
"""

import math
import os
import numpy as np

P = 128
NCORES = 8
NCH = 4         # table chunks (gather source chunks / AG pipeline stages)
GW = 8          # dst tiles per window

LAST_RESULT = None  # stash of BassKernelResults for test harness


# ----------------------------------------------------------------------------
# host-side graph preprocessing
# ----------------------------------------------------------------------------

def _host_prep(x, edge_index):
    N = x.shape[0]
    E = edge_index.shape[1]
    assert N % NCORES == 0
    npc = N // NCORES
    NT0 = (npc + P - 1) // P
    NT = ((NT0 + NCH - 1) // NCH) * NCH        # tiles per core, mult of NCH
    NPCP = NT * P
    SPC = NPCP // NCH                          # slots per chunk per core
    CH = NCORES * SPC                          # global rows per chunk
    TPC = NT // NCH                            # tiles per chunk per core
    NW = (NT + GW - 1) // GW
    assert CH <= 32512

    src = np.asarray(edge_index[0], np.int64)
    dst = np.asarray(edge_index[1], np.int64)
    deg = np.bincount(dst, minlength=N).astype(np.float32) + 1.0
    dinv = (1.0 / np.sqrt(deg.astype(np.float64))).astype(np.float32)

    order = np.argsort(-deg, kind="stable")
    ranks = np.empty(N, np.int64)
    ranks[order] = np.arange(N)
    core_of = ranks % NCORES
    r = ranks // NCORES
    # deal core-ranks round-robin over tiles -> balanced tiles
    slot_of = (r % NT) * P + (r // NT)
    newid = (slot_of // SPC) * CH + core_of * SPC + (slot_of % SPC)

    c_e = core_of[dst]
    l_e = slot_of[dst]
    t_e = l_e // P
    p_e = l_e % P
    w_e = t_e // GW
    gs = newid[src]
    m_e = gs // CH
    sv = (gs % CH).astype(np.int16)

    key = ((c_e * NW + w_e) * NCH + m_e) * NT + t_e
    eorder = np.argsort(key, kind="stable")
    cnt = np.bincount(key, minlength=NCORES * NW * NCH * NT)
    offs = np.zeros(len(cnt) + 1, np.int64)
    offs[1:] = np.cumsum(cnt)
    sv_s = sv[eorder]
    pv_s = p_e[eorder].astype(np.int64)

    calls = []                       # per (window, chunk) gather+S-matmul call
    win_calls = [[] for _ in range(NW)]
    window_mms = {}                  # w -> {t: [(ci, g, J), ...]}
    icol = 0
    scol = 0
    for w in range(NW):
        t0, t1 = w * GW, min((w + 1) * GW, NT)
        tl = list(range(t0, t1))
        ntl = len(tl)
        wm = {t: [] for t in tl}
        window_mms[w] = wm
        for m in range(NCH):
            segs = np.zeros((NCORES, ntl), np.int64)
            for ti, t in enumerate(tl):
                for c in range(NCORES):
                    segs[c, ti] = cnt[((c * NW + w) * NCH + m) * NT + t]
            tot = segs.sum(axis=1)
            clen = int(-(-int(tot.max()) // 16) * 16)
            if clen == 0:
                continue
            G = (clen + P - 1) // P
            starts = np.zeros((NCORES, ntl + 1), np.int64)
            starts[:, 1:] = np.cumsum(segs, axis=1)
            blocks = []
            ncols = 0
            ci = len(calls)
            for g in range(G):
                glo, ghi = g * P, (g + 1) * P
                for ti, t in enumerate(tl):
                    lo = np.maximum(starts[:, ti], glo)
                    hi = np.minimum(starts[:, ti + 1], ghi)
                    if not (lo < hi).any():
                        continue
                    J = scol + ncols
                    blocks.append((g, t, J))
                    wm[t].append((ci, g, J))
                    ncols += 1
            calls.append(dict(w=w, m=m, clen=clen, G=G, icol=icol, scol=scol,
                              ncols=ncols, blocks=blocks, starts=starts,
                              tl=tl))
            win_calls[w].append(ci)
            icol += clen // 16
            scol += ncols
    NMM = scol
    ICOLS = icol

    idx_flat = np.zeros((NCORES, ICOLS * 16), np.int16)
    spv = np.full((NCORES, P, NMM), -1.0, np.float16)
    for cl in calls:
        w, m, tl = cl["w"], cl["m"], cl["tl"]
        starts = cl["starts"]
        for c in range(NCORES):
            for ti, t in enumerate(tl):
                k = ((c * NW + w) * NCH + m) * NT + t
                a, b = int(offs[k]), int(offs[k + 1])
                n = b - a
                if n == 0:
                    continue
                pos = int(starts[c, ti]) + np.arange(n)
                idx_flat[c, cl["icol"] * 16 + pos] = sv_s[a:b]
        for (g, t, J) in cl["blocks"]:
            ti = t - tl[0]
            glo, ghi = g * P, (g + 1) * P
            for c in range(NCORES):
                k = ((c * NW + w) * NCH + m) * NT + t
                a = int(offs[k])
                sa, sb = int(starts[c, ti]), int(starts[c, ti + 1])
                lo, hi = max(sa, glo), min(sb, ghi)
                if lo >= hi:
                    continue
                rows = np.arange(lo, hi) - glo
                vals = pv_s[a + (lo - sa): a + (hi - sa)]
                spv[c, rows, J] = vals.astype(np.float16)

    # dense one-hot S blocks (streamed from DRAM per layer)
    smat = np.zeros((NCORES, P, NMM * P), np.float16)
    for c in range(NCORES):
        e_idx, J_idx = np.where(spv[c] >= 0)
        cols = J_idx * P + spv[c][e_idx, J_idx].astype(np.int64)
        smat[c, e_idx, cols] = 1.0

    # wrap indices: position j of a call -> partition j%16, col icol + j//16
    idxs = np.zeros((NCORES, P, ICOLS), np.int16)
    for c in range(NCORES):
        for cl in calls:
            cols = cl["clen"] // 16
            seg = idx_flat[c, cl["icol"] * 16:(cl["icol"] + cols) * 16]
            idxs[c, :16, cl["icol"]:cl["icol"] + cols] = \
                seg.reshape(cols, 16).T
        idxs[c] = np.tile(idxs[c, :16], (8, 1))

    dinv_t = np.zeros((NCORES, P, NT), np.float32)
    dinv_t[core_of, slot_of % P, slot_of // P] = dinv

    meta = dict(N=N, E=E, npc=npc, NT=NT, NPCP=NPCP, SPC=SPC, CH=CH, TPC=TPC,
                NW=NW, calls=calls, win_calls=win_calls,
                window_mms=window_mms, NMM=NMM, ICOLS=ICOLS,
                core_of=core_of, slot_of=slot_of)
    return meta, idxs, smat, dinv_t


# ----------------------------------------------------------------------------
# bass program
# ----------------------------------------------------------------------------

def _build_program(meta, DIN, DH, trace_sim=False):
    import concourse.bass as bass
    import concourse.bacc as bacc
    import concourse.tile as tile
    import concourse.mybir as mybir
    from concourse.masks import make_identity

    f16 = mybir.dt.float16
    f32 = mybir.dt.float32
    f8 = mybir.dt.float8e4
    i16 = mybir.dt.int16
    Relu = mybir.ActivationFunctionType.Relu
    Copy = mybir.ActivationFunctionType.Copy

    NT, NPCP, SPC, CH, TPC, NW = (meta["NT"], meta["NPCP"], meta["SPC"],
                                  meta["CH"], meta["TPC"], meta["NW"])
    KD = DIN // P
    calls = meta["calls"]
    win_calls = meta["win_calls"]
    window_mms = meta["window_mms"]
    Gmax = max(cl["G"] for cl in calls)
    smax = max(cl["ncols"] for cl in calls)

    nc = bacc.Bacc("TRN2", target_bir_lowering=False, debug=False,
                   num_devices=NCORES, num_swdge_queues=4)

    xT = nc.declare_dram_parameter("xT", [P, KD * NPCP], f16, isOutput=False)
    w1 = nc.declare_dram_parameter("w1", [P, KD * DH], f16, isOutput=False)
    w2 = nc.declare_dram_parameter("w2", [P, DH], f16, isOutput=False)
    w3 = nc.declare_dram_parameter("w3", [P, DH], f16, isOutput=False)
    wc = nc.declare_dram_parameter("wc", [P, 1], f16, isOutput=False)
    bias_p = nc.declare_dram_parameter("biases", [P, 4], f32, isOutput=False)
    dinv_p = nc.declare_dram_parameter("dinv", [P, NT], f32, isOutput=False)
    idxs_p = nc.declare_dram_parameter("idxs", [P, meta["ICOLS"]], i16,
                                       isOutput=False)
    smat_p = nc.declare_dram_parameter("smat", [P, meta["NMM"] * P], f16,
                                       isOutput=False)
    out_p = nc.declare_dram_parameter("out", [NPCP, 1], f32, isOutput=True)

    with tile.TileContext(nc, trace_sim=trace_sim) as tc:
        with tc.tile_pool(name="const", bufs=1) as cpool, \
             tc.tile_pool(name="dram", bufs=1, space="DRAM") as dpool, \
             tc.tile_pool(name="psum_w", bufs=2, space="PSUM") as wpsp, \
             tc.tile_pool(name="psum_t", bufs=2, space="PSUM") as tpsp, \
             tc.tile_pool(name="psum_a", bufs=4, space="PSUM") as apsp, \
             tc.tile_pool(name="gb", bufs=6) as gpool, \
             tc.tile_pool(name="sm", bufs=5) as spool, \
             tc.tile_pool(name="post", bufs=3) as ppool:

            w1s = cpool.tile([P, KD * DH], f16)
            w2s = cpool.tile([P, DH], f16)
            w3s = cpool.tile([P, DH], f16)
            wcs = cpool.tile([P, 1], f16)
            biases = cpool.tile([P, 4], f32)
            dinvs = cpool.tile([P, NT], f32)
            idxss = cpool.tile([P, meta["ICOLS"]], i16)
            ident = cpool.tile([P, P], f16)
            taba = cpool.tile([P, NT * DH], f16)
            tabb = cpool.tile([P, NT * DH], f16)
            outb = cpool.tile([P, NT], f32)

            for sbuf_t, dram_t in ((w1s, w1), (w2s, w2), (w3s, w3), (wcs, wc),
                                   (biases, bias_p), (dinvs, dinv_p),
                                   (idxss, idxs_p)):
                nc.sync.dma_start(out=sbuf_t[:], in_=dram_t[:])
            make_identity(nc, ident[:])
            # SWDGE warm-up gather (rings/queues init)
            with tc.tile_pool(name="warm", bufs=1) as wpool:
                dummy = wpool.tile([P, P], f16)
                nc.gpsimd.dma_gather(
                    out_ap=dummy[:].rearrange("p (g d) -> p g d", g=1),
                    in_ap=xT[:].rearrange("p (n d) -> (p n) d", d=P),
                    idxs_ap=idxss[:, :8],
                    num_idxs=P, num_idxs_reg=P, elem_size=DH,
                    single_packet=False)
            for i in range(6):
                z = gpool.tile([P, Gmax * P], f16, tag="gbuf", name=f"z{i}")
                nc.vector.memset(z[:], 0.0)

            agins = {}
            tbls = {}
            for ln in (1, 2, 3):
                agins[ln] = [dpool.tile([SPC, DH], f16, name=f"agin{ln}_{h}")
                             for h in range(NCH)]
                tbls[ln] = [dpool.tile([CH, DH], f16, addr_space="Shared",
                                       name=f"tbl{ln}_{h}")
                            for h in range(NCH)]

            def store_ag(tab, ln, c):
                agin, tbl = agins[ln][c], tbls[ln][c]
                nc.sync.dma_start(
                    out=agin[:].rearrange("(t p) d -> p t d", p=P),
                    in_=tab[:, c * TPC * DH:(c + 1) * TPC * DH]
                        .rearrange("p (t d) -> p t d", d=DH))
                nc.gpsimd.collective_compute(
                    "AllGather", mybir.AluOpType.bypass,
                    ins=[agin.opt()], outs=[tbl.opt()],
                    replica_groups=[list(range(NCORES))])

            # ---------------- phase 1: T1 = dinv * (X @ W1) ----------------
            XSL = 5
            assert TPC % XSL == 0
            with tc.tile_pool(name="xt", bufs=3) as xpool:
                for t0x in range(0, NT, XSL):
                    t1x = t0x + XSL
                    xts = xpool.tile([P, KD * XSL * P], f16, tag="xts",
                                     name=f"x{t0x}")
                    nc.sync.dma_start(
                        out=xts[:].rearrange("p (k q) -> p k q", k=KD),
                        in_=xT[:].rearrange("p (k n) -> p k n", k=KD)
                             [:, :, t0x * P:t1x * P])
                    for t in range(t0x, t1x):
                        ps = wpsp.tile([P, DH], f32, tag="wps", name=f"d{t}")
                        for k in range(KD):
                            nc.tensor.matmul(
                                out=ps[:],
                                lhsT=xts[:, (k * XSL + (t - t0x)) * P:
                                         (k * XSL + (t - t0x) + 1) * P],
                                rhs=w1s[:, k * DH:(k + 1) * DH],
                                start=(k == 0), stop=(k == KD - 1),
                                skip_group_check=True)
                        nc.scalar.mul(out=taba[:, t * DH:(t + 1) * DH],
                                      in_=ps[:], mul=dinvs[:, t:t + 1])
                        if (t + 1) % TPC == 0:
                            store_ag(taba, 1, t // TPC)

            layer_cfg = [
                (1, taba, 0, w2s, tabb, 2),
                (2, tabb, 1, w3s, taba, 3),
                (3, taba, 2, None, None, None),
            ]

            for li, (tln, tprev, bi, wnext, tnext, nextln) in \
                    enumerate(layer_cfg):
                for w in range(NW):
                    t0, t1 = w * GW, min((w + 1) * GW, NT)
                    tiles = list(range(t0, t1))
                    wcl = win_calls[w]
                    gbufs = {}
                    sbufs = {}
                    for ci in wcl:
                        cl = calls[ci]
                        G = cl["G"]
                        gb = gpool.tile([P, Gmax * P], f16, tag="gbuf",
                                        name=f"gb{li}_{ci}")
                        nc.gpsimd.dma_gather(
                            out_ap=gb[:, :G * P].rearrange(
                                "p (g d) -> p g d", g=G),
                            in_ap=tbls[tln][cl["m"]][:, :],
                            idxs_ap=idxss[:, cl["icol"]:
                                          cl["icol"] + cl["clen"] // 16],
                            num_idxs=cl["clen"], num_idxs_reg=cl["clen"],
                            elem_size=DH, single_packet=False,
                            queue_num=cl["m"])
                        gbufs[ci] = gb
                        ncols = cl["ncols"]
                        sb = spool.tile([P, smax * P], f16, tag="smat",
                                        name=f"sm{li}_{ci}")
                        nc.sync.dma_start(
                            out=sb[:, :ncols * P],
                            in_=smat_p[:, cl["scol"] * P:
                                       (cl["scol"] + ncols) * P])
                        sbufs[ci] = sb

                    wm = window_mms[w]
                    # per-tile accumulate + post, tile-major (PSUM
                    # accumulation chains must not interleave within a bank)
                    for ti, t in enumerate(tiles):
                        aps = apsp.tile([P, P], f32, tag="agg",
                                        name=f"ap{li}_{t}")
                        # self-loop term: identity matmul on own table rows
                        nc.tensor.matmul(
                            out=aps[:], lhsT=ident[:],
                            rhs=tprev[:, t * DH:(t + 1) * DH],
                            start=True, stop=(len(wm[t]) == 0),
                            skip_group_check=True)
                        for j, (ci, g, J) in enumerate(wm[t]):
                            cl = calls[ci]
                            nc.tensor.matmul(
                                out=aps[:],
                                lhsT=sbufs[ci][:, (J - cl["scol"]) * P:
                                               (J - cl["scol"] + 1) * P],
                                rhs=gbufs[ci][:, g * P:(g + 1) * P],
                                start=False, stop=(j == len(wm[t]) - 1),
                                skip_group_check=True)
                        # ---- post ----
                        tmp = ppool.tile([P, DH], f16, tag="tmp",
                                         name=f"tp{li}_{t}")
                        nc.scalar.mul(out=tmp[:], in_=aps[:],
                                      mul=dinvs[:, t:t + 1])
                        tps = tpsp.tile([P, P], f16, tag="tps",
                                        name=f"tt{li}_{t}")
                        nc.tensor.transpose(out=tps[:], in_=tmp[:],
                                            identity=ident[:])
                        rt = ppool.tile([P, P], f16, tag="rt",
                                        name=f"rt{li}_{t}")
                        nc.scalar.activation(out=rt[:], in_=tps[:], func=Relu,
                                             bias=biases[:, bi:bi + 1],
                                             scale=1.0)
                        if wnext is not None:
                            wp = wpsp.tile([P, DH], f32, tag="wps",
                                           name=f"wp{li}_{t}")
                            nc.tensor.matmul(out=wp[:], lhsT=rt[:], rhs=wnext,
                                             start=True, stop=True,
                                             skip_group_check=True)
                            nc.scalar.mul(out=tnext[:, t * DH:(t + 1) * DH],
                                          in_=wp[:], mul=dinvs[:, t:t + 1])
                            if (t + 1) % TPC == 0:
                                store_ag(tnext, nextln, t // TPC)
                        else:
                            wp = wpsp.tile([P, DH], f32, tag="wps",
                                           name=f"wo{li}_{t}")
                            nc.tensor.matmul(out=wp[:, :1], lhsT=rt[:],
                                             rhs=wcs[:],
                                             start=True, stop=True,
                                             skip_group_check=True)
                            nc.scalar.activation(out=outb[:, t:t + 1],
                                                 in_=wp[:, :1], func=Copy,
                                                 bias=float(0.0), scale=1.0)
                            # bc added on host side (scalar)

            nc.sync.dma_start(
                out=out_p[:].rearrange("(t p) o -> p t o", p=P),
                in_=outb[:].unsqueeze(2))

    nc.compile()
    return nc


# ----------------------------------------------------------------------------
# entry point
# ----------------------------------------------------------------------------

def kernel(x, edge_index, W1, b1, W2, b2, W3, b3, Wc, bc):
    global LAST_RESULT
    from concourse.bass_utils import run_bass_kernel_spmd

    x = np.asarray(x)
    N, DIN = x.shape
    DH = np.asarray(W1).shape[1]
    DH2 = np.asarray(W3).shape[1]

    meta, idxs, smat, dinv_t = _host_prep(x, edge_index)
    NT, NPCP = meta["NT"], meta["NPCP"]
    KD = DIN // P

    W3p = np.zeros((DH, DH), np.float32); W3p[:, :DH2] = np.asarray(W3)
    b3p = np.zeros(DH, np.float32); b3p[:DH2] = np.asarray(b3)
    Wcp = np.zeros((DH, 1), np.float32); Wcp[:DH2, :] = np.asarray(Wc)

    biases = np.zeros((P, 4), np.float32)
    biases[:, 0] = np.asarray(b1, np.float32)
    biases[:, 1] = np.asarray(b2, np.float32)
    biases[:, 2] = b3p
    biases[:, 3] = np.float32(np.asarray(bc).reshape(-1)[0])

    w1_sb = np.asarray(W1).reshape(KD, P, DH).transpose(1, 0, 2).reshape(
        P, KD * DH)
    common = dict(
        w1=w1_sb.astype(np.float16),
        w2=np.asarray(W2).astype(np.float16),
        w3=W3p.astype(np.float16),
        wc=Wcp.astype(np.float16),
        biases=biases,
    )

    core_of, slot_of = meta["core_of"], meta["slot_of"]
    xTc = np.zeros((NCORES, DIN, NPCP), np.float16)
    xf = x.astype(np.float16)
    for c in range(NCORES):
        sel = core_of == c
        xTc[c][:, slot_of[sel]] = xf[sel].T
    xTc = xTc.reshape(NCORES, KD, P, NPCP).transpose(0, 2, 1, 3).reshape(
        NCORES, P, KD * NPCP)

    in_maps = []
    for c in range(NCORES):
        m = dict(common)
        m["xT"] = np.ascontiguousarray(xTc[c])
        m["dinv"] = np.ascontiguousarray(dinv_t[c])
        m["idxs"] = np.ascontiguousarray(idxs[c])
        m["smat"] = np.ascontiguousarray(smat[c])
        in_maps.append(m)

    nc = _build_program(meta, DIN, DH)
    trace = os.environ.get("GCN_TRACE", "") == "1"
    res = run_bass_kernel_spmd(nc, in_maps, list(range(NCORES)), trace=trace)
    LAST_RESULT = res

    bc0 = np.float32(np.asarray(bc).reshape(-1)[0])
    outc = np.stack([res.results[c]["out"][:, 0] for c in range(NCORES)])
    y = (outc[core_of, slot_of] + bc0).astype(np.float32).reshape(N, 1)
    return y
